# revision 5
# baseline (speedup 1.0000x reference)
"""Trainium2 Bass kernel for nn_AttnBlock (GroupNorm + dense spatial attention).

Reference math (B=2, H=W=C=96, GROUPS=32, fp32):
    hn = GroupNorm32 over dim1(H) of x[B,H,W,C]  (stats over (3,W,C) per group)
    q/k/v = hn @ W* + b*;  scores = (q @ k^T)/sqrt(C) over HW=9216 per batch
    out = x + softmax(scores) @ v @ Wp + bp

Sharding (8 cores): core = (b, qc), b = core//4, qc = core%4. Each core holds
the full batch-b tensors (for K/V) plus its 2304-query-row chunk; the single
SPMD program needs no core id (all per-core differences enter via input data).

The kernel is exp-throughput-bound: 2304x9216 = 21.2M softmax exponentials
per core, and only ACT and DVE can read PSUM. The design splits every score
strip [128 keys, 1024 queries] between:
  - ACT: true Exp (scale folded), fp8e4 output           (~0.85us/strip)
  - DVE: Schraudolph fast-exp - one tensor_scalar computing
    int8(s*8*log2e*SCALE + 54.84) whose bits ARE the e4m3 encoding of
    2^(s*log2e*SCALE) (+-3% weight noise, cancels in the softmax ratio)
per EXP_PATTERN, tuned so both engines stay ~95% busy.

Attn@V runs as fp8e4 DoubleRow matmuls: key tiles are paired along a
free-dim axis of both operands ([128, 2, 97] vaug x [128, 2, mw] exp),
contracting 256 keys per pass at 2 output columns/cycle - half the PE time
of the bf16 version. vaug is stored fp8 at a 112-byte pair stride.

Other structure:
  - q/k projections folded on host: qT2 = (WqAug @ WkAugT)^T @ xsq_aug in
    one on-device matmul; kT/qT never materialize.
  - GroupNorm stats via masked matmuls into the oT psum banks; rsqrt =
    Quake seed + 1 Newton step (0.2% worst case, far below fp8 noise);
    scale/shift rows published as DRAM-bounce partition-broadcast DMAs,
    delta ranges only, per checkpoint (8/16/32 groups) so early chunks
    unblock long before the last stats land.
  - softmax denominator = vaug ones-column -> oT partition 96; postlude
    divides via a 1/rowsum DRAM-bounce broadcast (single-PSUM-operand
    limit) except the final block, which uses a PE ones-broadcast + copy
    to shorten the end-of-kernel serial chain.
  - PSUM: strips+prelude+postlude share one 3-deep [128,1024] tag
    (6 banks) - deep enough that an AA run in EXP_PATTERN cannot starve
    DVE - and the oT accumulator/stats share the other 2 banks.
  - queue discipline: input DMAs ordered critical-first across SP/ACT/Pool
    queues; Pool (gpsimd) runs the batch-side squares/scalings/memsets;
    single-partition row DMAs are split across queues and kept off the
    critical publish path.
"""

import numpy as np
import ml_dtypes

B, H, W, C = 2, 96, 96, 96
GROUPS = 32
EPS = 1e-5
HW = H * W                 # 9216
NCORES = 8
QCH = HW // 4              # 2304 query rows per core
GSPAN = HW // GROUPS       # 288 rows per group
QGROUPS = QCH // GSPAN     # 8 groups per query chunk
SCALE = float(C) ** -0.5
CA = C + 2                 # aug rows: 96=shiftRow, 97=ones
VA = C + 1                 # vaug cols: 96 = v, col 96 = ones
VPAD = 112                 # vaug tile stride (16-aligned for DoubleRow pairs)

LOG2E = 1.4426950408889634
A_DVE = 8.0 * LOG2E * SCALE     # fast-exp: bits = floor(s*A + B) as e4m3
B_DVE = 56.0 - 1.16             # 8*7 bias, -1.16 tuned for min spread

# exp engine pattern per strip index (A=ACT true exp, D=DVE fast exp);
# ratio tuned so ACT_busy ~= DVE_busy given DVE's other work. Runs of the
# same engine must stay <= 2: the strip psum rotation is only 3 deep, so a
# longer run starves the other engine.
EXP_PATTERN = "AAD" * 5 + "AD" * 11   # 21 ACT : 16 DVE, clustered

_compiled = {}


def _build_bass():
    import concourse.bass as bass
    import concourse.mybir as mybir
    import concourse.tile as tile

    # --- workaround: TRN2 allows one embedded sem-wait per instruction, but
    # TileContext piles every outstanding DMA-queue wait onto one tail drain.
    import bass_rust

    def _split_drain_and_barrier(self, tick_clock, wait_clock):
        nc = self.nc
        drain_inst = nc.sync.drain()
        wait_clock.add_sem_waits(
            drain_inst.ins, bass_rust.ScopedClock({None: tick_clock.global_clock})
        )
        si = drain_inst.ins.sync_info
        waits = list(si.on_wait) if si is not None and si.on_wait else []
        if len(waits) > 1:
            si.on_wait = waits[:1]
            for w in waits[1:]:
                extra = nc.sync.drain()
                esi = extra.ins.sync_info
                if esi is None:
                    extra.ins.sync_info = bass_rust.SyncInfo(on_wait=[w], on_update=[])
                else:
                    esi.on_wait = [w]
        nc.all_engine_barrier()
        assert self.sems is not None
        popped = nc._tile_sem_poison_stack.pop()
        assert popped is self._sem_poison
        nc.clear_and_free_semaphores(list(self.sems.allocated().values()))
        nc.all_engine_barrier()

    tile.TileContext._drain_and_barrier = _split_drain_and_barrier

    def _split_multiwaits(nc):
        """TRN2 ISA allows one embedded sem-wait per instruction; Tile's
        sem-assignment sometimes attaches several. Hoist extras onto
        engine-NOPs spliced immediately before the instruction."""
        n_split = 0
        for f in nc.m.functions:
            for bb in f.blocks:
                out = []
                changed = False
                for inst in bb.instructions:
                    si = getattr(inst, "sync_info", None)
                    if si is not None and si.on_wait and len(si.on_wait) > 1:
                        waits = list(si.on_wait)
                        for w in waits[:-1]:
                            n_split += 1
                            nop = bass_rust.InstNoOp(
                                name=f"WSPLIT-{n_split}", ins=[], outs=[]
                            )
                            nop.engine = inst.engine
                            nop.sync_info = bass_rust.SyncInfo(
                                on_wait=[w], on_update=[]
                            )
                            nc.register_instruction(nop)
                            out.append(nop)
                        si.on_wait = waits[-1:]
                        changed = True
                    out.append(inst)
                if changed:
                    bb.instructions = out
        return n_split

    f32 = mybir.dt.float32
    bf16 = mybir.dt.bfloat16
    f8e4 = mybir.dt.float8e4
    i8 = mybir.dt.int8
    AF = mybir.ActivationFunctionType
    ALU = mybir.AluOpType
    AX = mybir.AxisListType
    DR = mybir.MatmulPerfMode.DoubleRow

    nc = bass.Bass()

    xbT16 = nc.dram_tensor("xbT16", [C, HW], bf16, kind="ExternalInput")
    xqT16 = nc.dram_tensor("xqT16", [C, QCH], bf16, kind="ExternalInput")
    xqT = nc.dram_tensor("xqT", [C, QCH], f32, kind="ExternalInput")
    gRow = nc.dram_tensor("gRow", [GROUPS, GSPAN], f32, kind="ExternalInput")
    bRow = nc.dram_tensor("bRow", [GROUPS, GSPAN], f32, kind="ExternalInput")
    gRowQ = nc.dram_tensor("gRowQ", [QGROUPS, GSPAN], f32, kind="ExternalInput")
    bRowQ = nc.dram_tensor("bRowQ", [QGROUPS, GSPAN], f32, kind="ExternalInput")
    WfoldD = nc.dram_tensor("WfoldD", [CA, CA], bf16, kind="ExternalInput")
    WvAug = nc.dram_tensor("WvAug", [CA, VA], bf16, kind="ExternalInput")
    Wp = nc.dram_tensor("Wp", [C, C], bf16, kind="ExternalInput")
    bp = nc.dram_tensor("bp", [C, 1], f32, kind="ExternalInput")
    masksBD = nc.dram_tensor("masksBD", [C, GROUPS * GROUPS], bf16,
                             kind="ExternalInput")
    masksLD = nc.dram_tensor("masksLD", [C, QGROUPS * QGROUPS], bf16,
                             kind="ExternalInput")
    outT = nc.dram_tensor("outT", [C, QCH], f32, kind="ExternalOutput")
    # internal DRAM bounce for scale rows (partition-broadcast DMA source);
    # one tensor per stats checkpoint (DRAM dep tracking is whole-tensor)
    scRowD = [nc.dram_tensor(f"scRowD{j}", [HW], bf16) for j in range(3)]
    scRowQD = nc.dram_tensor("scRowQD", [QCH], bf16)
    # 1/rowsum bounce: one tensor per m-block (whole-tensor DRAM dep tracking)
    rrD = [nc.dram_tensor(f"rrD{j}", [1024], bf16) for j in range(3)]

    NTILES = HW // 128       # 72 key tiles
    NPAIRS = NTILES // 2     # 36 DoubleRow key pairs
    MBLocks = [1024, 1024, 256]
    CHK = 1152               # 4 whole groups; prelude pipelines at this grain

    with tile.TileContext(nc) as tc:
        import contextlib

        with contextlib.ExitStack() as ctx:
            consts = ctx.enter_context(tc.tile_pool(name="consts", bufs=1))
            big = ctx.enter_context(tc.tile_pool(name="big", bufs=1))
            # ALL psum comes from these two pools; disjoint regions: strips +
            # prelude/postlude share one 3-deep tag (6 banks), oT (2 banks).
            # 3-deep strips give the exp engines a 2-strip lookahead so an
            # AA/DD run in EXP_PATTERN doesn't starve the other engine.
            sps = ctx.enter_context(tc.tile_pool(name="sps", bufs=3, space="PSUM"))
            ops = ctx.enter_context(tc.tile_pool(name="ot_ps", bufs=1, space="PSUM"))
            sqp = ctx.enter_context(tc.tile_pool(name="sq_sb", bufs=2))
            stb = ctx.enter_context(tc.tile_pool(name="stat_sb", bufs=2))
            esb = ctx.enter_context(tc.tile_pool(name="exp_sb", bufs=6))
            psg = ctx.enter_context(tc.tile_pool(name="pstage", bufs=3))
            osb = ctx.enter_context(tc.tile_pool(name="post_sb", bufs=2))

            # ---- big SBUF tensors (declared early; loads get top priority) --
            xb16 = big.tile([C, HW], bf16)       # raw bf16 x (stats + scaling)
            xq16 = big.tile([C, QCH], bf16)
            xqT_s = big.tile([C, QCH], f32)      # fp32 x kept only for residual

            xsb = big.tile([CA, HW], bf16)       # [x*scale; shift; 1] batch
            xsq = big.tile([CA, QCH], bf16)      # local query chunk
            xsq_ones = xsq[C : C + 2, :]
            xsb_ones = xsb[C : C + 2, :]
            ones96b = consts.tile([VA, C], bf16)

            wf_t = consts.tile([CA, CA], bf16)
            wva_t = consts.tile([CA, VA], bf16)
            wp_t = consts.tile([C, C], bf16)
            bp_t = consts.tile([C, 1], f32)
            masksB = consts.tile([C, GROUPS * GROUPS], bf16)
            masksL = consts.tile([C, QGROUPS * QGROUPS], bf16)

            gt_l = consts.tile([QGROUPS, GSPAN], f32, name="grow_L")
            bt_l = consts.tile([QGROUPS, GSPAN], f32, name="brow_L")
            gt_b = consts.tile([GROUPS, GSPAN], f32, name="grow_B")
            bt_b = consts.tile([GROUPS, GSPAN], f32, name="brow_B")
            grow = {"L": (gt_l, bt_l), "B": (gt_b, bt_b)}

            # transfers serialize per issuing queue, so order each queue by
            # when its payloads unblock compute: the local (query) chain
            # first on sync, batch chunks spread over all three queues,
            # postlude-only payloads (xqT_s, wp, bp) last on gpsimd.
            def chunk(dst, src, i):
                sl = slice(i * CHK, (i + 1) * CHK)
                return dst[:, sl], src[:, sl]

            for eng, dst, src in [
                (nc.sync, *chunk(xq16, xqT16, 0)),
                (nc.sync, masksL, masksLD[:, :]),
                (nc.sync, gt_l, gRowQ[:, :]),
                (nc.sync, bt_l, bRowQ[:, :]),
                (nc.sync, *chunk(xq16, xqT16, 1)),
                (nc.sync, wf_t, WfoldD[:, :]),
                (nc.scalar, *chunk(xb16, xbT16, 1)),
                (nc.scalar, *chunk(xb16, xbT16, 3)),
                (nc.gpsimd, *chunk(xb16, xbT16, 5)),
                (nc.gpsimd, *chunk(xb16, xbT16, 7)),
                (None, None, None),  # ones-row memsets slot (see below)
                (nc.sync, *chunk(xb16, xbT16, 0)),
                (nc.sync, masksB, masksBD[:, :]),
                (nc.sync, gt_b, gRow[:, :]),
                (nc.sync, bt_b, bRow[:, :]),
                (nc.sync, *chunk(xb16, xbT16, 2)),
                (nc.sync, *chunk(xb16, xbT16, 4)),
                (nc.sync, *chunk(xb16, xbT16, 6)),
                (nc.sync, wva_t, WvAug[:, :]),
            ]:
                if eng is None:
                    # aug ones rows, emitted here so they precede the
                    # postlude-only loads in the gpsimd queue but follow the
                    # urgent xb16 chunks (gpsimd memset needs 32-aligned
                    # partition starts; shift-row DMAs overwrite row 96).
                    # xsb's ones row is per-chunk in the stats loop: one
                    # 7.7us memset here would jam the Pool queue ahead of
                    # the critical finish_side publishes.
                    nc.gpsimd.memset(xsq_ones, 1.0)
                    nc.gpsimd.memset(ones96b[C : C + 1, :], 1.0)
                    continue
                eng.dma_start(out=dst, in_=src)

            # dummy exp to trigger the ACT table load while DMAs run
            dumm = consts.tile([1, 8], f32)
            nc.vector.memset(dumm, 0.0)
            dumm2 = consts.tile([1, 8], f32)
            nc.scalar.activation(dumm2, dumm, AF.Exp)


            stats_acc = {
                "L": consts.tile([QGROUPS, 2], f32, name="accL"),
                "B": consts.tile([GROUPS, 2], f32, name="accB"),
            }
            nc.vector.memset(stats_acc["L"], 0.0)
            nc.vector.memset(stats_acc["B"], 0.0)

            # ---- big SBUF tensors ----
            qT2 = big.tile([CA, QCH], bf16)      # (WqAug @ WkAugT)^T @ xsq
            vaug = big.tile([128, NTILES * VPAD], f8e4)
            oTr = big.tile([VA, QCH], bf16)      # attn@v evac; row 96 = rowsum
            rrB = big.tile([VA, QCH], bf16)      # 1/rowsum staged on part 96


            CNT = 1.0 / (GSPAN * C)

            def stats_chunk(x16, key, i):
                """Colsums of groups 4i..4i+3 -> rows 4i+j of the side's
                packed [ngroups, 2] stats accumulator (lane-aligned)."""
                masks, ng = (masksL, QGROUPS) if key == "L" else (masksB, GROUPS)
                acc = stats_acc[key]
                chunk = x16[:, i * CHK : (i + 1) * CHK]
                sq = sqp.tile([C, CHK], bf16, tag="sq", name="sq")
                sq_eng = nc.vector if key == "L" else nc.gpsimd
                sq_eng.tensor_mul(sq, chunk, chunk)
                # stats use the oT accumulator banks only: sharing the strip
                # tag would serialize the batch stats behind the local
                # prelude through the 3-deep sp rotation
                ts_ = ops.tile([VA, 1024], f32, tag="oT", name="ts")
                ps_s = ts_[0:ng, 0:GSPAN]
                ps_q = ts_[0:ng, 512 : 512 + GSPAN]
                for j in range(4):
                    g = 4 * i + j
                    sspan = slice(j * GSPAN, (j + 1) * GSPAN)
                    mk = masks[:, g * ng : (g + 1) * ng]
                    nc.tensor.matmul(
                        ps_s, mk, chunk[:, sspan], start=(j == 0), stop=(j == 3)
                    )
                    nc.tensor.matmul(
                        ps_q, mk, sq[:, sspan], start=(j == 0), stop=(j == 3)
                    )
                red = stb.tile([GROUPS, 2], f32, tag="red", name="red")[:ng]
                both = ts_[0:ng, :].rearrange("p (a s) -> p a s", a=2)[:, :, 0:GSPAN]
                nc.vector.tensor_reduce(red, both, axis=AX.X, op=ALU.add)
                nc.vector.tensor_add(acc, acc, red)

            fin_prev = {"L": 0, "B": 0}

            def finish_side(key, srowd, xs_t, k=None):
                """All per-group scalar math for one side in [ngroups]-wide
                ops: mean/var, DVE-only rsqrt (Quake seed + 3 Newton steps),
                scale/shift rows, and the two cast-DMAs that publish them."""
                ng = QGROUPS if key == "L" else GROUPS
                if k is None:
                    k = ng
                g_t, b_t = grow[key]
                g_t, b_t = g_t[:k], b_t[:k]
                acc = stats_acc[key][:k]
                st = stb.tile([GROUPS, 12], f32, tag="st", name="st")[:k]
                mean, ex2 = st[:, 0:1], st[:, 1:2]
                msq, var = st[:, 2:3], st[:, 3:4]
                veps, ti = st[:, 4:5], st[:, 5:6]
                ya, yb = st[:, 6:7], st[:, 7:8]
                rstd = st[:, 8:9]
                nc.vector.tensor_scalar_mul(mean, in0=acc[:, 0:1], scalar1=CNT)
                nc.vector.tensor_scalar_mul(ex2, in0=acc[:, 1:2], scalar1=CNT)
                nc.vector.tensor_mul(msq, mean, mean)
                nc.vector.tensor_sub(var, ex2, msq)
                nc.vector.tensor_scalar_add(veps, in0=var, scalar1=EPS)
                i32 = mybir.dt.int32
                nc.vector.tensor_scalar(
                    out=ti.bitcast(i32), in0=veps.bitcast(i32),
                    scalar1=1, scalar2=-1, op0=ALU.arith_shift_right,
                    op1=ALU.bitwise_xor,
                )
                nc.vector.tensor_scalar_add(
                    rstd.bitcast(i32), in0=ti.bitcast(i32), scalar1=0x5F3759E0
                )
                # one Newton step: Quake seed err ~3.4% -> ~0.2%, far below
                # the fp8/fast-exp noise floor
                for _ in range(1):
                    nc.vector.tensor_mul(ya, rstd, rstd)
                    nc.vector.tensor_mul(yb, ya, veps)
                    nc.vector.tensor_scalar(
                        out=yb, in0=yb, scalar1=-0.5, scalar2=1.5,
                        op0=ALU.mult, op1=ALU.add,
                    )
                    nc.vector.tensor_mul(rstd, rstd, yb)
                sc32 = stb.tile([GROUPS, GSPAN], f32, tag="sc", name="sc32")[:k]
                nc.vector.tensor_scalar_mul(sc32, in0=g_t, scalar1=rstd)
                ms32 = stb.tile([GROUPS, GSPAN], f32, tag="ms", name="ms32")[:k]
                nc.vector.tensor_scalar_mul(ms32, in0=sc32, scalar1=mean)
                sh32 = stb.tile([GROUPS, GSPAN], f32, tag="sh", name="sh32")[:k]
                nc.vector.tensor_sub(sh32, b_t, ms32)
                sh16 = stb.tile([GROUPS, GSPAN], bf16, tag="sh16", name="sh16")[:k]
                nc.vector.tensor_copy(sh16, sh32)
                # publish only the delta rows kp:k (earlier checkpoints own
                # their prefix); single-partition row DMAs are slow, so
                # halving their bytes matters
                kp = fin_prev[key]
                fin_prev[key] = k
                nd = k - kp
                nc.gpsimd.dma_start(
                    out=srowd[kp * GSPAN : k * GSPAN].rearrange(
                        "(g s) -> g s", s=GSPAN
                    ),
                    in_=sc32[kp:k],
                )
                sh_eng = nc.scalar if kp == 0 else nc.sync
                sh_eng.dma_start(
                    out=xs_t[C : C + 1, kp * GSPAN : k * GSPAN].rearrange(
                        "p (g s) -> p g s", g=nd
                    ),
                    in_=sh16[kp:k].rearrange("g (a s) -> g a s", a=1),
                )

            def scaled_chunk(x16, xs_t, srowd, i):
                """xs = x * scaleB; scaleB lands via a broadcast DMA whose
                DRAM source repeats the scale row across all partitions."""
                sl = slice(i * CHK, (i + 1) * CHK)
                scb = sqp.tile([C, CHK], bf16, tag="scb", name="scb")
                bcast_src = bass.AP(
                    tensor=srowd, offset=i * CHK, ap=[[0, C], [1, CHK]]
                )
                nc.sync.dma_start(out=scb, in_=bcast_src)
                # batch side on Pool: slow (2.4us vs 0.7) but off the DVE
                # exp engine, and the chunk cadence (~11us) dwarfs it; the
                # local side is latency-critical and DVE is idle then
                eng = nc.vector if x16 is xq16 else nc.gpsimd
                eng.tensor_mul(xs_t[0:C, sl], x16[:, sl], scb)

            def emit_vaug(i):
                """v-projection for chunk i's 9 key tiles, during the
                stats/prelude phase: the ACT evacs land in ACT's early idle
                window instead of stalling mid-run C-DR pairs."""
                t0 = i * 9
                for base, cnt in [(0, 5), (5, 4)]:
                    tv = sps.tile([128, 512], f32, tag="sp", name="tv")
                    for j in range(cnt):
                        nc.tensor.matmul(
                            tv[:, j * VA : (j + 1) * VA],
                            xsb[
                                :,
                                (t0 + base + j) * 128 : (t0 + base + j + 1) * 128,
                            ],
                            wva_t,
                            start=True, stop=True,
                        )
                    # evac to the padded fp8 vaug layout (strided dst)
                    nc.scalar.activation(
                        vaug[
                            :, (t0 + base) * VPAD : (t0 + base + cnt) * VPAD
                        ].rearrange("p (t m) -> p t m", t=cnt)[:, :, 0:VA],
                        tv[:, 0 : cnt * VA].rearrange(
                            "p (t m) -> p t m", t=cnt
                        ),
                        AF.Copy,
                    )

            # ---- local (query) prelude ----
            # qT2 = (WqAug @ WkAugT)^T @ xsq_aug in one matmul (weights
            # folded on host); evacuations on ACT (idle until exps start).
            for i in range(2):
                stats_chunk(xq16, "L", i)
            finish_side("L", scRowQD, xsq)
            for i in range(2):
                scaled_chunk(xq16, xsq, scRowQD, i)
                for off, w in [(0, 512), (512, 512), (1024, 128)]:
                    sl = slice(i * CHK + off, i * CHK + off + w)
                    tq = sps.tile([128, 512], f32, tag="sp", name="tq")
                    nc.tensor.matmul(
                        tq[0:CA, 0:w], wf_t, xsq[:, sl], start=True, stop=True
                    )
                    nc.scalar.activation(qT2[:, sl], tq[0:CA, 0:w], AF.Copy)

            # ---- batch prelude: stats -> xsb -> vaug per 1152-chunk ----
            for i in range(8):
                stats_chunk(xb16, "B", i)
                nc.gpsimd.memset(xsb_ones[:, i * CHK : (i + 1) * CHK], 1.0)
                if i == 1:
                    finish_side("B", scRowD[0], xsb, k=8)
                    for j in (0, 1):
                        scaled_chunk(xb16, xsb, scRowD[0], j)
                        emit_vaug(j)
                elif i == 3:
                    finish_side("B", scRowD[1], xsb, k=16)
                    for j in (2, 3):
                        scaled_chunk(xb16, xsb, scRowD[1], j)
                elif i == 7:
                    finish_side("B", scRowD[2], xsb, k=GROUPS)
                    for j in range(4, 8):
                        scaled_chunk(xb16, xsb, scRowD[2], j)

            # postlude-only loads, emitted here so they sit behind the
            # whole stats/scaling chain in the gpsimd queue
            for dst, src_ in [
                (xqT_s[:, 0:CHK], xqT[:, 0:CHK]),
                (xqT_s[:, CHK : 2 * CHK], xqT[:, CHK : 2 * CHK]),
                (wp_t, Wp[:, :]),
                (bp_t, bp[:, :]),
            ]:
                nc.gpsimd.dma_start(out=dst, in_=src_)

            exp_ctr = [0]

            def mb_open(mw):
                return {
                    "oT": ops.tile([VA, 1024], f32, tag="oT", name="oT"),
                    "pend": [], "next": 0,
                    "spb": 1024 // mw,
                    "halves": [(h, min(512, mw - h)) for h in range(0, mw, 512)],
                    "ex_half": [None],   # pending pair tile (mw=1024 only)
                }

            def mb_emit(st, mo, mw, upto_tile):
                spb, halves = st["spb"], st["halves"]
                nst = NTILES // spb
                while st["next"] < nst and st["next"] * spb < upto_tile:
                    s = st["next"]
                    sp = sps.tile([128, 1024], f32, tag="sp", name="sp")
                    for j in range(spb):
                        t = s * spb + j
                        for h, hw_ in halves:
                            nc.tensor.matmul(
                                sp[:, j * mw + h : j * mw + h + hw_],
                                xsb[:, t * 128 : (t + 1) * 128],
                                qT2[:, mo + h : mo + h + hw_],
                                start=True, stop=True,
                            )
                    # exp strip -> fp8e4, rotating ACT / DVE / Pool producers
                    eng_c = EXP_PATTERN[exp_ctr[0] % len(EXP_PATTERN)]
                    exp_ctr[0] += 1
                    if spb == 1:
                        # pair tile [128, 2048]: halves = strips 2P, 2P+1
                        if st["ex_half"][0] is None:
                            ex = esb.tile([128, 2048], f8e4, tag="ex", name="ex")
                            st["ex_half"][0] = ex
                            dst = ex[:, 0:1024]
                            pair_done = False
                        else:
                            ex = st["ex_half"][0]
                            dst = ex[:, 1024:2048]
                            st["ex_half"][0] = None
                            pair_done = True
                    else:
                        # spb=4 (mw=256): one strip tile holds 2 full pairs
                        ex = esb.tile([128, 1024], f8e4, tag="ex", name="ex")
                        dst = ex[:, 0:1024]
                        pair_done = True
                    if eng_c == "A":
                        nc.scalar.activation(
                            dst, sp[:, : spb * mw], AF.Exp, scale=SCALE
                        )
                    elif eng_c == "D":
                        nc.vector.tensor_scalar(
                            out=dst.bitcast(i8), in0=sp[:, : spb * mw],
                            scalar1=A_DVE, scalar2=B_DVE,
                            op0=ALU.mult, op1=ALU.add,
                        )
                    else:
                        # Pool can't read PSUM: bounce the raw scores to
                        # SBUF by DMA (SP queue), fast-exp on gpsimd
                        stage = psg.tile([128, 1024], f32, tag="pst",
                                         name="stage")
                        nc.sync.dma_start(
                            out=stage[:, : spb * mw], in_=sp[:, : spb * mw]
                        )
                        nc.gpsimd.tensor_scalar(
                            out=dst.bitcast(i8), in0=stage[:, : spb * mw],
                            scalar1=A_DVE, scalar2=B_DVE,
                            op0=ALU.mult, op1=ALU.add,
                        )
                    if pair_done:
                        st["pend"].append((s, ex))
                        # keep a few pairs pending: the first C-DR of a block
                        # waits on the oT slot (freed late), and a deeper
                        # backlog keeps it out of the PE queue head
                        if len(st["pend"]) > 3:
                            _mb_c(st, mo, mw)
                    st["next"] += 1

            def _mb_c(st, mo, mw):
                """DoubleRow attn@V for one ready pair group: contract 256
                keys per matmul (2 key-tiles in the free-dim pair axis)."""
                spb, halves = st["spb"], st["halves"]
                s_, ex_ = st["pend"].pop(0)
                if spb == 1:
                    P = s_ // 2  # pair index; ex_ = [128, 2048] strips (2P, 2P+1)
                    va = vaug[:, 2 * P * VPAD : (2 * P + 2) * VPAD].rearrange(
                        "p (i m) -> p i m", i=2
                    )[:, :, 0:VA]
                    exp_pair = ex_[:, :].rearrange("p (i n) -> p i n", i=2)
                    for h, hw_ in halves:
                        nc.tensor.matmul(
                            st["oT"][:, h : h + hw_],
                            va, exp_pair[:, :, h : h + hw_],
                            start=(P == 0), stop=(P == NPAIRS - 1),
                            perf_mode=DR,
                        )
                else:
                    # strip s_ covers tiles 4s..4s+3 = pairs 2s, 2s+1
                    for j in range(2):
                        P = 2 * s_ + j
                        va = vaug[:, 2 * P * VPAD : (2 * P + 2) * VPAD].rearrange(
                            "p (i m) -> p i m", i=2
                        )[:, :, 0:VA]
                        exp_pair = ex_[:, j * 2 * mw : (j + 1) * 2 * mw].rearrange(
                            "p (i n) -> p i n", i=2
                        )
                        nc.tensor.matmul(
                            st["oT"][:, 0:mw],
                            va, exp_pair,
                            start=(P == 0), stop=(P == NPAIRS - 1),
                            perf_mode=DR,
                        )

            mb_idx = [0]

            def mb_finish(st, mo, mw):
                while st["pend"]:
                    _mb_c(st, mo, mw)
                oT = st["oT"]
                rd = rrD[min(mb_idx[0], 2)]
                mb_idx[0] += 1
                final = mb_idx[0] == 3  # set after the increment above
                # one evac: channels 0..95 (o) + partition 96 (rowsum), bf16
                nc.vector.tensor_copy(oTr[:, mo : mo + mw], oT[0:VA, :mw])
                with nc.allow_low_precision(reason="softmax denom; bf16 ok"):
                    nc.vector.reciprocal(
                        rrB[C : C + 1, mo : mo + mw],
                        oTr[C : C + 1, mo : mo + mw],
                    )
                if not final:
                    # 1/rowsum -> DRAM -> partition-broadcast back: the
                    # scaled product then has a single PSUM operand (HW
                    # limit) with no ones-broadcast matmul
                    nc.sync.dma_start(
                        out=rd[0:mw].rearrange("(a n) -> a n", a=1),
                        in_=rrB[C : C + 1, mo : mo + mw],
                    )
                po = mo
                while po < mo + mw:
                    pw = min(512, mo + mw - po)
                    tc_ = sps.tile([128, 512], f32, tag="sp", name="tpost")
                    pp = tc_[0:C, 0:pw]
                    nc.tensor.matmul(
                        pp, wp_t, oTr[0:C, po : po + pw], start=True, stop=True
                    )
                    sc = osb.tile([C, 512], f32, tag="sc", name="sc")
                    if final:
                        # tail block: PE ones-broadcast + one copy beats the
                        # two-DMA roundtrip on the end-of-kernel latency path
                        tc2 = sps.tile([128, 512], f32, tag="sp", name="tpo2")
                        pr = tc2[0:C, 0:pw]
                        nc.tensor.matmul(
                            pr, ones96b[C : C + 1, :],
                            rrB[C : C + 1, po : po + pw],
                            start=True, stop=True, tile_position=(96, 0),
                        )
                        prs = osb.tile([C, 512], bf16, tag="rbc", name="prs")
                        nc.vector.tensor_copy(prs[:, :pw], pr)
                        nc.vector.tensor_mul(sc[:, :pw], pp, prs[:, :pw])
                    else:
                        rbc = osb.tile([C, 512], bf16, tag="rbc", name="rbc")
                        nc.sync.dma_start(
                            out=rbc[:, :pw],
                            in_=bass.AP(tensor=rd, offset=po - mo,
                                        ap=[[0, C], [1, pw]]),
                        )
                        nc.vector.tensor_mul(sc[:, :pw], pp, rbc[:, :pw])
                    ot = osb.tile([C, 512], f32, tag="ot", name="ot")
                    nc.vector.scalar_tensor_tensor(
                        out=ot[:, :pw], in0=sc[:, :pw], scalar=bp_t,
                        in1=xqT_s[:, po : po + pw],
                        op0=ALU.add, op1=ALU.add,
                    )
                    nc.sync.dma_start(out=outT[:, po : po + pw], in_=ot[:, :pw])
                    po += pw

            st0 = mb_open(1024)
            for i in range(8):
                if i >= 2:
                    emit_vaug(i)
                mb_emit(st0, 0, 1024, 9 * (i + 1))
            # bridge m-block boundaries: pre-emit the next block's first
            # strips before draining this block's tail, so ACT/DVE never
            # idle across the transition.
            st1 = mb_open(1024)
            mb_emit(st1, 1024, 1024, 18)
            mb_finish(st0, 0, 1024)
            mb_emit(st1, 1024, 1024, NTILES)
            st2 = mb_open(256)
            mb_emit(st2, 2048, 256, 40)
            mb_finish(st1, 1024, 1024)
            mb_emit(st2, 2048, 256, NTILES)
            mb_finish(st2, 2048, 256)

    _split_multiwaits(nc)
    return nc


def _prep_inputs(x, gamma, beta, Wq, bq, Wk, bk, Wv, bv, Wp, bp):
    bf16 = ml_dtypes.bfloat16
    f32 = np.float32

    x2 = np.ascontiguousarray(x.reshape(B, HW, C))
    gRow = np.repeat(np.asarray(gamma, f32), W).reshape(GROUPS, GSPAN)
    bRow = np.repeat(np.asarray(beta, f32), W).reshape(GROUPS, GSPAN)

    WvAug = np.zeros((CA, VA), f32)
    WvAug[:C, :C] = Wv
    WvAug[C, :C] = Wv.sum(axis=0)      # u_v: shiftRow coefficient
    WvAug[C + 1, :C] = bv
    WvAug[C + 1, C] = 1.0              # ones column -> softmax denominator

    def aug(Wm, bias):
        a = np.empty((CA, C), f32)
        a[:C] = Wm
        a[C] = Wm.sum(axis=0)
        a[C + 1] = bias
        return a

    WqAug = aug(np.asarray(Wq, f32), bq)
    WkAugT = np.ascontiguousarray(aug(np.asarray(Wk, f32), bk).T)
    # scores = (WqAug^T xq)^T (WkAug^T xk) = xq^T (WqAug WkAugT) xk:
    # fold both projections into one [CA, CA] weight applied to the q side
    Wfold = (WqAug.astype(np.float64) @ WkAugT.astype(np.float64)).astype(f32)

    masksB = np.zeros((C, GROUPS * GROUPS), f32)
    for g in range(GROUPS):
        masksB[:, g * GROUPS + g] = 1.0
    masksL = np.zeros((C, QGROUPS * QGROUPS), f32)
    for g in range(QGROUPS):
        masksL[:, g * QGROUPS + g] = 1.0

    in_maps = []
    for core in range(NCORES):
        b, qc = divmod(core, 4)
        xbT = np.ascontiguousarray(x2[b].T)
        xqT = np.ascontiguousarray(xbT[:, qc * QCH : (qc + 1) * QCH])
        in_maps.append({
            "xbT16": xbT.astype(bf16),
            "xqT16": xqT.astype(bf16),
            "xqT": xqT.astype(f32),
            "gRow": gRow,
            "bRow": bRow,
            "gRowQ": np.ascontiguousarray(gRow.reshape(4, QGROUPS, GSPAN)[qc]),
            "bRowQ": np.ascontiguousarray(bRow.reshape(4, QGROUPS, GSPAN)[qc]),
            "WfoldD": Wfold.astype(bf16),
            "WvAug": WvAug.astype(bf16), "Wp": Wp.astype(bf16),
            "bp": np.asarray(bp, f32).reshape(C, 1),
            "masksBD": masksB.astype(bf16),
            "masksLD": masksL.astype(bf16),
        })
    return in_maps


def _get_sharded_fn():
    """Build the 8-core shard_map callable once so repeated calls reuse the
    compiled NEFF executable."""
    if "fn" in _compiled:
        return _compiled["fn"]

    import jax
    import jax.numpy as jnp
    from jax.sharding import Mesh, PartitionSpec
    from jax.experimental.shard_map import shard_map
    import concourse.mybir as mybir
    from concourse.bass2jax import (
        _bass_exec_p, install_neuronx_cc_hook, partition_id_tensor
    )

    if "nc" not in _compiled:
        _compiled["nc"] = _build_bass()
    nc = _compiled["nc"]
    install_neuronx_cc_hook()

    pname = nc.partition_id_tensor.name if nc.partition_id_tensor else None
    in_names, out_names, out_avals = [], [], []
    for alloc in nc.m.functions[0].allocations:
        if not isinstance(alloc, mybir.MemoryLocationSet):
            continue
        name = alloc.memorylocations[0].name
        if alloc.kind == "ExternalInput":
            if name != pname:
                in_names.append(name)
        elif alloc.kind == "ExternalOutput":
            out_names.append(name)
            out_avals.append(
                jax.core.ShapedArray(
                    tuple(alloc.tensor_shape), mybir.dt.np(alloc.dtype)
                )
            )
    n_params = len(in_names)
    all_names = in_names + out_names
    if pname is not None:
        all_names = all_names + [pname]

    def _body(*args):
        operands = list(args)
        if pname is not None:
            operands.append(partition_id_tensor())
        outs = _bass_exec_p.bind(
            *operands,
            out_avals=tuple(out_avals),
            in_names=tuple(all_names),
            out_names=tuple(out_names),
            lowering_input_output_aliases=(),
            sim_require_finite=True,
            sim_require_nnan=True,
            nc=nc,
        )
        return tuple(outs)

    devices = jax.devices()[:NCORES]
    mesh = Mesh(np.asarray(devices), ("core",))
    sharded = jax.jit(
        shard_map(
            _body, mesh=mesh,
            in_specs=(PartitionSpec("core"),) * (n_params + len(out_names)),
            out_specs=(PartitionSpec("core"),) * len(out_names),
            check_rep=False,
        ),
        keep_unused=True,
    )

    from jax.sharding import NamedSharding

    shard = NamedSharding(mesh, PartitionSpec("core"))

    def put(in_maps):
        """Upload per-core inputs + zero outputs once; reuse across calls."""
        dev = [
            jax.device_put(
                np.concatenate(
                    [np.asarray(in_maps[c][nm]) for c in range(NCORES)], axis=0
                ),
                shard,
            )
            for nm in in_names
        ]
        dev += [
            jax.device_put(
                np.zeros((NCORES * a.shape[0], *a.shape[1:]), a.dtype), shard
            )
            for a in out_avals
        ]
        return dev

    def execute(dev_in):
        return sharded(*dev_in)

    def run(in_maps):
        out_arrs = execute(put(in_maps))
        return {
            nm: np.asarray(out_arrs[i]).reshape(NCORES, *out_avals[i].shape)
            for i, nm in enumerate(out_names)
        }

    _compiled["fn"] = (run, out_names, put, execute)
    _compiled["mkchain"] = (sharded, in_names, out_names, _body)
    return _compiled["fn"]


def kernel(x, gamma, beta, Wq, bq, Wk, bk, Wv, bv, Wp, bp):
    run = _get_sharded_fn()[0]
    in_maps = _prep_inputs(
        np.asarray(x, np.float32), gamma, beta, Wq, bq, Wk, bk, Wv, bv, Wp, bp
    )
    res = run(in_maps)["outT"]

    out = np.empty((B, HW, C), np.float32)
    for core in range(NCORES):
        b, qc = divmod(core, 4)
        out[b, qc * QCH : (qc + 1) * QCH, :] = res[core].T
    return out.reshape(B, H, W, C)
